# revision 42
# baseline (speedup 1.0000x reference)
"""Trainium2 kernel for nn_BpsMlp: KNN min-distance (B=64,N=1024 queries vs
M=4096 basis points) feeding a 4-layer MLP, data-parallel over batch across
8 NeuronCores.

Per core (8 batches = 8192 query rows):
  - distance phase: d2[q,m] accumulated exactly in fp32 PSUM via K=16
    augmented bf16 hi/lo matmuls (catastrophic-cancellation-free), four
    matmuls packed concurrently into the PE via tile_position row-groups.
    PSUM holds -d2 (pos_aug negated on host) so min-reduction is all MAX.
    Egress splits across the only two engines with a PSUM port (ScalarE
    casts 3424 values to fp16 SBUF, VectorE max-pairs the other 672
    against the copy); VectorE 2x-mode folds the fp16 stream one qtile
    deferred, and a fused tensor_scalar max-accum produces
    x_sb[:, t] = -min(d2). (GpSimd is unusable: no PSUM port, and TRN2
    walrus rejects TensorTensor and InstPool on the Pool engine.)
  - x = sqrt(max(d2min, 1e-12)) with one Newton refinement step.
  - MLP in fp16 (weights streamed to SBUF during the distance phase),
    h^T layout [hid-tile 128, batch 8], relu+bias on VectorE.
"""

import sys

sys.path.insert(0, "/opt/trn_rl_repo")

import numpy as np
import ml_dtypes

import concourse.bass as bass
import concourse.mybir as mybir
import concourse.tile as tile
from concourse.bass import ds, ts
from concourse.bass_utils import run_bass_kernel_spmd

BF16 = ml_dtypes.bfloat16
DT = mybir.dt
AF = mybir.ActivationFunctionType
OP = mybir.AluOpType

B, N, M = 64, 1024, 4096
HID, OUT = 2048, 512
NCORES = 8
BPC = B // NCORES            # batches per core
R = BPC * N                  # query rows per core (8192)
QT = R // 128                # q-tiles per core (64)
KAUG = 16                    # augmented contraction dim
MT_H = HID // 128            # hid tiles (16)
KT1 = N // 128               # L1 k-tiles (8)
KT2 = HID // 128             # L2/L3/L4 k-tiles (16)
MT_O = OUT // 128            # out tiles (4)

_cache = {}


def _split_hi_lo(v):
    vh = v.astype(BF16).astype(np.float32)
    vl = (v - vh).astype(BF16).astype(np.float32)
    return vh, vl


def _build_program():
    nc = bass.Bass()

    posT = nc.declare_dram_parameter("posT_aug", [128, R], DT.bfloat16, isOutput=False)
    basisA = nc.declare_dram_parameter("basis_aug", [128, M], DT.bfloat16, isOutput=False)
    w0 = nc.declare_dram_parameter("w0", [128, KT1 * HID], DT.float16, isOutput=False)
    w1 = nc.declare_dram_parameter("w1", [128, KT2 * HID], DT.float16, isOutput=False)
    w2 = nc.declare_dram_parameter("w2", [128, KT2 * HID], DT.float16, isOutput=False)
    w3 = nc.declare_dram_parameter("w3", [128, KT2 * OUT], DT.float16, isOutput=False)
    b0d = nc.declare_dram_parameter("b0t", [128, MT_H], DT.float32, isOutput=False)
    b1d = nc.declare_dram_parameter("b1t", [128, MT_H], DT.float32, isOutput=False)
    b2d = nc.declare_dram_parameter("b2t", [128, MT_H], DT.float32, isOutput=False)
    b3d = nc.declare_dram_parameter("b3t", [128, MT_O], DT.float32, isOutput=False)
    outT = nc.declare_dram_parameter("outT", [MT_O, 128, BPC], DT.float32, isOutput=True)

    with tile.TileContext(nc) as tc:
        with (
            tc.tile_pool(name="const", bufs=1) as const,
            tc.tile_pool(name="psum", bufs=2, space="PSUM") as psum,
            tc.tile_pool(name="cpp", bufs=2) as cpp,
            tc.tile_pool(name="drain", bufs=2) as drain,
            tc.tile_pool(name="junk", bufs=1) as junk,
            tc.tile_pool(name="posc", bufs=2) as posc,
        ):
            basis_sb = const.tile([128, M], DT.bfloat16)
            for j in range(8):
                nc.sync.dma_start(basis_sb[:, ts(j, M // 8)], basisA[:, ts(j, M // 8)])

            w0_sb = const.tile([128, KT1 * HID], DT.float16)
            w1_sb = const.tile([128, KT2 * HID], DT.float16)
            w2_sb = const.tile([128, KT2 * HID], DT.float16)
            w3_sb = const.tile([128, KT2 * OUT], DT.float16)
            b0_sb = const.tile([128, MT_H], DT.float32)
            b1_sb = const.tile([128, MT_H], DT.float32)
            b2_sb = const.tile([128, MT_H], DT.float32)
            b3_sb = const.tile([128, MT_O], DT.float32)

            x_sb = const.tile([128, QT], DT.float32)

            # touch Sqrt now so its 1.3us activation-table load happens under
            # the startup DMAs instead of between the distance and MLP phases
            warm = const.tile([128, 1], DT.float32)
            nc.vector.memset(warm[:], 1.0)
            nc.scalar.activation(warm[:], warm[:], AF.Sqrt)

            # ---- distance phase ----
            # pos/basis augmented rows replicated into 4 PE row-groups so the
            # four K=16 matmuls per unit run concurrently (tile_position).
            # Every d2 value crosses exactly one of the two PSUM read paths
            # (ScalarE 1.2 GHz / VectorE 0.96 GHz), which is the structural
            # drain floor; PSUM has no GpSimd port and one DVE read port.
            # MLP weight DMAs are spread across the blocks so the pos-chunk
            # prefetches never sit behind a deep weight backlog.
            wdmas = []
            for j in range(KT1):
                wdmas.append((w0_sb[:, ts(j, HID)], w0[:, ts(j, HID)]))
            for j in range(KT2):
                wdmas.append((w1_sb[:, ts(j, HID)], w1[:, ts(j, HID)]))
                wdmas.append((w2_sb[:, ts(j, HID)], w2[:, ts(j, HID)]))
                wdmas.append((w3_sb[:, ts(j, OUT)], w3[:, ts(j, OUT)]))
            wdmas.append((b0_sb[:], b0d[:]))
            wdmas.append((b1_sb[:], b1d[:]))
            wdmas.append((b2_sb[:], b2d[:]))
            wdmas.append((b3_sb[:], b3d[:]))
            wd_i = 0

            pos_tiles = {}

            def fold_block(t, cp_, s_):
                # Deferred fold of qtile t (issued during qtile t+1 so DVE
                # never stalls on ScalarE's second copy): 2x-mode max-folds
                # of the 2752 leftover copies and the 672 paired maxima,
                # then one fused tensor_scalar max-accum into x_sb[:, t].
                f2 = junk.tile([128, 1024], DT.float16, tag="f2", bufs=1)
                nc.vector.tensor_tensor(
                    f2[:, 688:1024], s_[:, 0:336], s_[:, 336:672], op=OP.max
                )
                f1 = junk.tile([128, 1376], DT.float16, tag="f1", bufs=1)
                nc.vector.tensor_tensor(
                    f1[:], cp_[:, 672:2048], cp_[:, 2048:3424], op=OP.max
                )
                nc.vector.tensor_tensor(f2[:, 0:688], f1[:, 0:688], f1[:, 688:1376], op=OP.max)
                f3 = junk.tile([128, 512], DT.float16, tag="f3", bufs=1)
                nc.vector.tensor_tensor(f3[:], f2[:, 0:512], f2[:, 512:1024], op=OP.max)
                fo = junk.tile([128, 1376], DT.float16, tag="f1", bufs=1)
                nc.vector.tensor_scalar(
                    fo[:, 0:512], f3[:], 1.0, None,
                    op0=OP.mult, op1=OP.max, accum_out=x_sb[:, t : t + 1],
                )

            def issue_chunk(c):
                pc_ = posc.tile([128, 128], DT.bfloat16, tag="posc")
                nc.sync.dma_start(pc_[:, 0:64], posT[:, ds(c * 128, 64)])
                nc.sync.dma_start(pc_[:, 64:128], posT[:, ds(c * 128 + 64, 64)])
                pos_tiles[c] = pc_

            issue_chunk(0)
            pending = None
            for t in range(QT):
                if t + 1 < QT:
                    issue_chunk(t + 1)
                if t % 8 == 0 and t >= 8:
                    # no weight traffic during the startup window (qtiles
                    # 0-7): a 512 KB weight chunk ahead of a pos-chunk DMA
                    # in the same queue stalls the PE for ~7us.
                    c8 = t // 8
                    n_issue = (len(wdmas) * c8) // (QT // 8 - 1) - wd_i
                    for _ in range(n_issue):
                        dst, src = wdmas[wd_i]
                        nc.sync.dma_start(dst, src)
                        wd_i += 1
                pos_chunk = pos_tiles[t]
                # PSUM holds -d2 (pos_aug negated on the host) so every
                # reduction is a MAX — required for GpSimd's pool (max-only).
                # tile A: m in [0,2048). Drained by ScalarE alone (fp16 cast
                # to SBUF), freeing its banks early for the next qtile.
                ptA = psum.tile([128, 2048], DT.float32, tag="psA", bufs=1)
                for j in range(4):
                    nc.tensor.matmul(
                        ptA[:, ts(j, 512)],
                        pos_chunk[32 * j : 32 * j + KAUG, 0:128],
                        basis_sb[32 * j : 32 * j + KAUG, ts(j, 512)],
                        tile_position=(32 * j, 0),
                    )
                cp = cpp.tile([128, 3424], DT.float16, tag="cp")
                nc.scalar.copy(cp[:, 0:2048], ptA[:])
                # tile B: m in [2048,4096).
                ptB = psum.tile([128, 2048], DT.float32, tag="psB", bufs=1)
                for j in range(4):
                    nc.tensor.matmul(
                        ptB[:, ts(j, 512)],
                        pos_chunk[32 * j : 32 * j + KAUG, 0:128],
                        basis_sb[32 * j : 32 * j + KAUG, ds(2048 + j * 512, 512)],
                        tile_position=(32 * j, 0),
                    )
                # Balanced 2-engine drain (PSUM egress is the wall: only DVE
                # ~1.04ns/el and Act ~0.83ns/el have PSUM ports; GpSimd has
                # no PSUM port and TRN2 walrus rejects every GpSimd free-axis
                # compute op): Act egresses A (2048) + B[0:1376]; DVE
                # egresses B[1376:] paired against A's copy (one max level
                # for free); the fp16 fold runs on DVE in 2x mode, deferred
                # one qtile (fold_block).
                nc.scalar.copy(cp[:, 2048:3424], ptB[:, 0:1376])
                s = junk.tile([128, 672], DT.float16, tag="s", bufs=2)
                nc.vector.tensor_tensor(s[:], ptB[:, 1376:2048], cp[:, 0:672], op=OP.max)
                if pending is not None:
                    fold_block(*pending)
                pending = (t, cp, s)
            fold_block(*pending)

            # ---- x = sqrt(max(d2,1e-12)), one Newton step ----
            # x_sb holds -min(d2); negate while clamping.
            # temps live in recycled junk-pool slots (SBUF is tight).
            xc = junk.tile([128, 512], DT.float32, tag="f2", bufs=1)
            xc = xc[:, 0:QT]
            nc.vector.tensor_scalar(
                xc[:], x_sb[:], -1.0, 1e-12, op0=OP.mult, op1=OP.max
            )
            y0 = junk.tile([128, 256], DT.float32, tag="f3", bufs=1)
            y0 = y0[:, 0:QT]
            nc.scalar.activation(y0[:], xc[:], AF.Sqrt)
            ry = junk.tile([128, 688], DT.float32, tag="f1", bufs=1)
            ry = ry[:, 0:QT]
            nc.vector.reciprocal(ry[:], y0[:])
            t1 = junk.tile([128, QT], DT.float32, tag="s", bufs=2)
            nc.vector.tensor_mul(t1[:], xc[:], ry[:])
            t2 = junk.tile([128, 512], DT.float32, tag="f2", bufs=1)
            t2 = t2[:, 0:QT]
            nc.vector.tensor_add(t2[:], y0[:], t1[:])
            xbf = junk.tile([128, 512], DT.float16, tag="f3", bufs=1)
            xbf = xbf[:, 0:QT]
            nc.vector.tensor_scalar_mul(xbf[:], t2[:], 0.5)

            # ---- MLP (h^T layout: [hid-tile 128, batch 8]) ----
            xg = xbf[:].rearrange("p (b t) -> p t b", t=KT1)
            zero_t = const.tile([128, BPC], DT.float16)
            nc.vector.memset(zero_t[:], 0.0)

            def layer(in_view, w_sb, b_sb, n_kt, n_mt, act_relu, out_dtype):
                # One small psum tile per mt-group: consecutive groups
                # ping-pong the two pool slots, so the relu's PSUM read never
                # serializes against the next group's matmuls (Tile tracks
                # PE-write vs DVE-read conflicts at whole-tile granularity).
                hout = drain.tile([128, n_mt * BPC], out_dtype, tag="h" + str(n_mt))
                for mt in range(n_mt):
                    pt = psum.tile(
                        [128, BPC], DT.float32,
                        tag="psA" if mt % 2 == 0 else "psB", bufs=1,
                    )
                    for kt in range(n_kt):
                        nc.tensor.matmul(
                            pt[:],
                            w_sb[:, ds(kt * n_mt * 128 + mt * 128, 128)],
                            in_view[:, kt, :],
                            start=(kt == 0),
                            stop=(kt == n_kt - 1),
                        )
                    if act_relu:
                        # relu(psum + bias) on VectorE (idle during MLP)
                        nc.vector.scalar_tensor_tensor(
                            hout[:, ds(mt * BPC, BPC)],
                            pt[:],
                            b_sb[:, mt : mt + 1],
                            zero_t[:],
                            op0=OP.add,
                            op1=OP.max,
                        )
                    else:
                        nc.scalar.activation(
                            hout[:, ds(mt * BPC, BPC)],
                            pt[:],
                            AF.Identity,
                            bias=b_sb[:, mt : mt + 1],
                        )
                return hout

            h1 = layer(xg, w0_sb, b0_sb, KT1, MT_H, True, DT.float16)
            h1v = h1[:].rearrange("p (t b) -> p t b", b=BPC)
            h2 = layer(h1v, w1_sb, b1_sb, KT2, MT_H, True, DT.float16)
            h2v = h2[:].rearrange("p (t b) -> p t b", b=BPC)
            h3 = layer(h2v, w2_sb, b2_sb, KT2, MT_H, True, DT.float16)
            h3v = h3[:].rearrange("p (t b) -> p t b", b=BPC)
            h4 = layer(h3v, w3_sb, b3_sb, KT2, MT_O, False, DT.float32)

            for mt in range(MT_O):
                nc.sync.dma_start(outT[mt], h4[:, ds(mt * BPC, BPC)])

    # InstTensorTensorReduce is an extended-inst InstISA subclass; populate
    # its .instr bytes (raw Bass skips this pass -> "ISA wrong length").
    mybir.codegen_inst_isa_subclasses(nc)
    _split_multi_waits(nc)
    return nc


def _split_multi_waits(nc, max_waits=1):
    """neuronx-cc walrus rejects instructions with >1 sync wait; hoist extras
    onto nofuse NOPs just before, on the same engine. Extended-ISA structs
    (InstISA subclasses, e.g. TensorTensorReduce) can't carry ANY sync in
    walrus codegen (setupSyncWait<UNKNOWN_STRUCT>), so for those hoist all
    waits onto a NOP before and all updates onto a NOP after — same-engine
    program order preserves the semantics."""
    ctr = 0
    for f in nc.m.functions:
        for bb in f.blocks:
            new_insts = []
            for ins in bb.instructions:
                limit = 0 if isinstance(ins, mybir.InstISA) else max_waits
                si = getattr(ins, "sync_info", None)
                if si is not None and si.on_wait and len(si.on_wait) > limit:
                    waits = list(si.on_wait)
                    if limit:
                        extra, keep = waits[:-limit], waits[-limit:]
                    else:
                        extra, keep = waits, []
                    step = max(max_waits, 1)
                    for i in range(0, len(extra), step):
                        ctr += 1
                        new_insts.append(
                            mybir.InstNoOp(
                                name=f"waitsplit-{ctr}",
                                engine=ins.engine,
                                sync_info=mybir.SyncInfo(
                                    on_wait=extra[i : i + step], on_update=[]
                                ),
                                bass_nofuse=True,
                            )
                        )
                    si.on_wait = keep
                new_insts.append(ins)
                if isinstance(ins, mybir.InstISA) and si is not None and si.on_update:
                    updates = list(si.on_update)
                    si.on_update = []
                    ctr += 1
                    new_insts.append(
                        mybir.InstNoOp(
                            name=f"updsplit-{ctr}",
                            engine=ins.engine,
                            sync_info=mybir.SyncInfo(on_wait=[], on_update=updates),
                            bass_nofuse=True,
                        )
                    )
            bb.instructions[:] = new_insts


def _prep_inputs(pos, basis, W0, b0, W1, b1, W2, b2, W3, b3):
    pos = np.asarray(pos, dtype=np.float32)
    basis = np.asarray(basis, dtype=np.float32)

    bh, bl = _split_hi_lo(basis)  # [M,3]
    q2 = (basis * basis).sum(-1)
    q2h, q2l = _split_hi_lo(q2)
    ones_m = np.ones(M, np.float32)
    basis_aug = np.zeros((16, M), np.float32)
    basis_aug[0:3] = bh.T
    basis_aug[3:6] = bh.T
    basis_aug[6:9] = bl.T
    basis_aug[9:12] = bl.T
    basis_aug[12] = ones_m
    basis_aug[13] = ones_m
    basis_aug[14] = q2h
    basis_aug[15] = q2l
    # replicate into the 4 PE row-groups (partitions 32g..32g+15)
    basis_rep = np.zeros((128, M), np.float32)
    for g in range(4):
        basis_rep[32 * g : 32 * g + 16] = basis_aug
    basis_rep = basis_rep.astype(BF16)

    def pos_aug_for_core(c):
        p = pos[c * BPC : (c + 1) * BPC].reshape(R, 3)
        a = -2.0 * p
        ah, al = _split_hi_lo(a)
        p2 = (p * p).sum(-1)
        p2h, p2l = _split_hi_lo(p2)
        ones_r = np.ones(R, np.float32)
        pa = np.zeros((16, R), np.float32)
        pa[0:3] = ah.T
        pa[3:6] = al.T
        pa[6:9] = ah.T
        pa[9:12] = al.T
        pa[12] = p2h
        pa[13] = p2l
        pa[14] = ones_r
        pa[15] = ones_r
        pa = -pa  # PSUM accumulates -d2 so the on-chip reduction can be MAX
        pa_rep = np.zeros((128, R), np.float32)
        for g in range(4):
            pa_rep[32 * g : 32 * g + 16] = pa
        return pa_rep.astype(BF16)

    def pack_w(W, n_kt, n_out):
        return (
            np.asarray(W, np.float32)
            .reshape(n_kt, 128, n_out)
            .transpose(1, 0, 2)
            .reshape(128, n_kt * n_out)
            .astype(np.float16)
        )

    common = {
        "basis_aug": basis_rep,
        "w0": pack_w(W0, KT1, HID),
        "w1": pack_w(W1, KT2, HID),
        "w2": pack_w(W2, KT2, HID),
        "w3": pack_w(W3, KT2, OUT),
        "b0t": np.asarray(b0, np.float32).reshape(MT_H, 128).T.copy(),
        "b1t": np.asarray(b1, np.float32).reshape(MT_H, 128).T.copy(),
        "b2t": np.asarray(b2, np.float32).reshape(MT_H, 128).T.copy(),
        "b3t": np.asarray(b3, np.float32).reshape(MT_O, 128).T.copy(),
    }
    in_maps = []
    for c in range(NCORES):
        m = dict(common)
        m["posT_aug"] = pos_aug_for_core(c)
        in_maps.append(m)
    return in_maps


def kernel(pos, basis, W0, b0, W1, b1, W2, b2, W3, b3, _trace=False):
    if "nc" not in _cache:
        _cache["nc"] = _build_program()
    nc = _cache["nc"]
    in_maps = _prep_inputs(pos, basis, W0, b0, W1, b1, W2, b2, W3, b3)
    res = run_bass_kernel_spmd(nc, in_maps, list(range(NCORES)), trace=_trace)
    _cache["last_result"] = res
    out = np.empty((B, OUT), np.float32)
    for c in range(NCORES):
        o = np.asarray(res.results[c]["outT"])  # [MT_O, 128, BPC]
        out[c * BPC : (c + 1) * BPC] = o.transpose(2, 0, 1).reshape(BPC, OUT)
    return out



# revision 44
# speedup vs baseline: 1.0342x; 1.0342x over previous
"""Trainium2 kernel for nn_BpsMlp: KNN min-distance (B=64,N=1024 queries vs
M=4096 basis points) feeding a 4-layer MLP, data-parallel over batch across
8 NeuronCores.

Per core (8 batches = 8192 query rows):
  - distance phase: d2[q,m] accumulated exactly in fp32 PSUM via K=16
    augmented bf16 hi/lo matmuls (catastrophic-cancellation-free), four
    matmuls packed concurrently into the PE via tile_position row-groups.
    PSUM holds -d2 (pos_aug negated on host) so min-reduction is all MAX.
    Egress splits across the only two engines with a PSUM port (ScalarE
    casts 3424 values to fp16 SBUF, VectorE max-pairs the other 672
    against the copy); VectorE 2x-mode folds the fp16 stream one qtile
    deferred, and a fused tensor_scalar max-accum produces
    x_sb[:, t] = -min(d2). (GpSimd is unusable: no PSUM port, and TRN2
    walrus rejects TensorTensor and InstPool on the Pool engine.)
  - x = sqrt(max(d2min, 1e-12)) with one Newton refinement step.
  - MLP in fp16 (weights streamed to SBUF during the distance phase),
    h^T layout [hid-tile 128, batch 8], relu+bias on VectorE.
"""

import sys

sys.path.insert(0, "/opt/trn_rl_repo")

import numpy as np
import ml_dtypes

import concourse.bass as bass
import concourse.mybir as mybir
import concourse.tile as tile
from concourse.bass import ds, ts
from concourse.bass_utils import run_bass_kernel_spmd

BF16 = ml_dtypes.bfloat16
DT = mybir.dt
AF = mybir.ActivationFunctionType
OP = mybir.AluOpType

B, N, M = 64, 1024, 4096
HID, OUT = 2048, 512
NCORES = 8
BPC = B // NCORES            # batches per core
R = BPC * N                  # query rows per core (8192)
QT = R // 128                # q-tiles per core (64)
KAUG = 16                    # augmented contraction dim
MT_H = HID // 128            # hid tiles (16)
KT1 = N // 128               # L1 k-tiles (8)
KT2 = HID // 128             # L2/L3/L4 k-tiles (16)
MT_O = OUT // 128            # out tiles (4)

_cache = {}


def _split_hi_lo(v):
    vh = v.astype(BF16).astype(np.float32)
    vl = (v - vh).astype(BF16).astype(np.float32)
    return vh, vl


def _build_program():
    nc = bass.Bass()

    posT = nc.declare_dram_parameter("posT_aug", [128, R], DT.bfloat16, isOutput=False)
    basisA = nc.declare_dram_parameter("basis_aug", [128, M], DT.bfloat16, isOutput=False)
    w0 = nc.declare_dram_parameter("w0", [128, KT1 * HID], DT.float16, isOutput=False)
    w1 = nc.declare_dram_parameter("w1", [128, KT2 * HID], DT.float16, isOutput=False)
    w2 = nc.declare_dram_parameter("w2", [128, KT2 * HID], DT.float16, isOutput=False)
    w3 = nc.declare_dram_parameter("w3", [128, KT2 * OUT], DT.float16, isOutput=False)
    b0d = nc.declare_dram_parameter("b0t", [128, MT_H], DT.float32, isOutput=False)
    b1d = nc.declare_dram_parameter("b1t", [128, MT_H], DT.float32, isOutput=False)
    b2d = nc.declare_dram_parameter("b2t", [128, MT_H], DT.float32, isOutput=False)
    b3d = nc.declare_dram_parameter("b3t", [128, MT_O], DT.float32, isOutput=False)
    outT = nc.declare_dram_parameter("outT", [MT_O, 128, BPC], DT.float32, isOutput=True)

    with tile.TileContext(nc) as tc:
        with (
            tc.tile_pool(name="const", bufs=1) as const,
            tc.tile_pool(name="psum", bufs=2, space="PSUM") as psum,
            tc.tile_pool(name="cpp", bufs=2) as cpp,
            tc.tile_pool(name="drain", bufs=2) as drain,
            tc.tile_pool(name="junk", bufs=1) as junk,
            tc.tile_pool(name="posc", bufs=2) as posc,
        ):
            basis_sb = const.tile([128, M], DT.bfloat16)
            for j in range(8):
                nc.sync.dma_start(basis_sb[:, ts(j, M // 8)], basisA[:, ts(j, M // 8)])

            w0_sb = const.tile([128, KT1 * HID], DT.float16)
            w1_sb = const.tile([128, KT2 * HID], DT.float16)
            w2_sb = const.tile([128, KT2 * HID], DT.float16)
            w3_sb = const.tile([128, KT2 * OUT], DT.float16)
            b0_sb = const.tile([128, MT_H], DT.float32)
            b1_sb = const.tile([128, MT_H], DT.float32)
            b2_sb = const.tile([128, MT_H], DT.float32)
            b3_sb = const.tile([128, MT_O], DT.float32)

            x_sb = const.tile([128, QT], DT.float32)

            # touch Sqrt now so its 1.3us activation-table load happens under
            # the startup DMAs instead of between the distance and MLP phases
            warm = const.tile([128, 1], DT.float32)
            nc.vector.memset(warm[:], 1.0)
            nc.scalar.activation(warm[:], warm[:], AF.Sqrt)

            # ---- distance phase ----
            # pos/basis augmented rows replicated into 4 PE row-groups so the
            # four K=16 matmuls per unit run concurrently (tile_position).
            # Every d2 value crosses exactly one of the two PSUM read paths
            # (ScalarE 1.2 GHz / VectorE 0.96 GHz), which is the structural
            # drain floor; PSUM has no GpSimd port and one DVE read port.
            # MLP weight DMAs are spread across the blocks so the pos-chunk
            # prefetches never sit behind a deep weight backlog.
            wdmas = []
            for j in range(KT1):
                wdmas.append((w0_sb[:, ts(j, HID)], w0[:, ts(j, HID)]))
            for j in range(KT2):
                wdmas.append((w1_sb[:, ts(j, HID)], w1[:, ts(j, HID)]))
                wdmas.append((w2_sb[:, ts(j, HID)], w2[:, ts(j, HID)]))
                wdmas.append((w3_sb[:, ts(j, OUT)], w3[:, ts(j, OUT)]))
            # biases are 64 B each — issue up front, they can't clog a queue
            nc.sync.dma_start(b0_sb[:], b0d[:])
            nc.sync.dma_start(b1_sb[:], b1d[:])
            nc.sync.dma_start(b2_sb[:], b2d[:])
            nc.sync.dma_start(b3_sb[:], b3d[:])
            wd_i = 0

            pos_tiles = {}

            def fold_block(t, cp_, s_):
                # Deferred fold of qtile t (issued during qtile t+1 so DVE
                # never stalls on ScalarE's second copy): 2x-mode max-folds
                # of the 2752 leftover copies and the 672 paired maxima,
                # then one fused tensor_scalar max-accum into x_sb[:, t].
                f2 = junk.tile([128, 1024], DT.float16, tag="f2", bufs=1)
                nc.vector.tensor_tensor(
                    f2[:, 688:1024], s_[:, 0:336], s_[:, 336:672], op=OP.max
                )
                f1 = junk.tile([128, 1376], DT.float16, tag="f1", bufs=1)
                nc.vector.tensor_tensor(
                    f1[:], cp_[:, 672:2048], cp_[:, 2048:3424], op=OP.max
                )
                nc.vector.tensor_tensor(f2[:, 0:688], f1[:, 0:688], f1[:, 688:1376], op=OP.max)
                f3 = junk.tile([128, 512], DT.float16, tag="f3", bufs=1)
                nc.vector.tensor_tensor(f3[:], f2[:, 0:512], f2[:, 512:1024], op=OP.max)
                fo = junk.tile([128, 1376], DT.float16, tag="f1", bufs=1)
                nc.vector.tensor_scalar(
                    fo[:, 0:512], f3[:], 1.0, None,
                    op0=OP.mult, op1=OP.max, accum_out=x_sb[:, t : t + 1],
                )

            def issue_chunk(c):
                pc_ = posc.tile([128, 128], DT.bfloat16, tag="posc")
                nc.sync.dma_start(pc_[:, 0:64], posT[:, ds(c * 128, 64)])
                nc.sync.dma_start(pc_[:, 64:128], posT[:, ds(c * 128 + 64, 64)])
                pos_tiles[c] = pc_

            issue_chunk(0)
            pending = None
            for t in range(QT):
                if t + 1 < QT:
                    issue_chunk(t + 1)
                if t >= 8 and wd_i < len(wdmas):
                    # exactly one weight DMA per qtile, none during startup:
                    # a 512 KB weight chunk queued ahead of a pos-chunk DMA
                    # stalls the PE ~7us, and block-issuing weights loads 8
                    # consecutive queues at once so the round-robin rotation
                    # wraps onto a still-draining queue. One per qtile gives
                    # each queue ~56us to drain its 23us transfer.
                    dst, src = wdmas[wd_i]
                    nc.sync.dma_start(dst, src)
                    wd_i += 1
                pos_chunk = pos_tiles[t]
                # PSUM holds -d2 (pos_aug negated on the host) so every
                # reduction is a MAX — required for GpSimd's pool (max-only).
                # tile A: m in [0,2048). Drained by ScalarE alone (fp16 cast
                # to SBUF), freeing its banks early for the next qtile.
                ptA = psum.tile([128, 2048], DT.float32, tag="psA", bufs=1)
                for j in range(4):
                    nc.tensor.matmul(
                        ptA[:, ts(j, 512)],
                        pos_chunk[32 * j : 32 * j + KAUG, 0:128],
                        basis_sb[32 * j : 32 * j + KAUG, ts(j, 512)],
                        tile_position=(32 * j, 0),
                    )
                cp = cpp.tile([128, 3424], DT.float16, tag="cp")
                nc.scalar.copy(cp[:, 0:2048], ptA[:])
                # tile B: m in [2048,4096).
                ptB = psum.tile([128, 2048], DT.float32, tag="psB", bufs=1)
                for j in range(4):
                    nc.tensor.matmul(
                        ptB[:, ts(j, 512)],
                        pos_chunk[32 * j : 32 * j + KAUG, 0:128],
                        basis_sb[32 * j : 32 * j + KAUG, ds(2048 + j * 512, 512)],
                        tile_position=(32 * j, 0),
                    )
                # Balanced 2-engine drain (PSUM egress is the wall: only DVE
                # ~1.04ns/el and Act ~0.83ns/el have PSUM ports; GpSimd has
                # no PSUM port and TRN2 walrus rejects every GpSimd free-axis
                # compute op): Act egresses A (2048) + B[0:1376]; DVE
                # egresses B[1376:] paired against A's copy (one max level
                # for free); the fp16 fold runs on DVE in 2x mode, deferred
                # one qtile (fold_block).
                nc.scalar.copy(cp[:, 2048:3424], ptB[:, 0:1376])
                s = junk.tile([128, 672], DT.float16, tag="s", bufs=2)
                nc.vector.tensor_tensor(s[:], ptB[:, 1376:2048], cp[:, 0:672], op=OP.max)
                if pending is not None:
                    fold_block(*pending)
                pending = (t, cp, s)
            fold_block(*pending)

            # ---- x = sqrt(max(d2,1e-12)), one Newton step ----
            # x_sb holds -min(d2); negate while clamping.
            # temps live in recycled junk-pool slots (SBUF is tight).
            xc = junk.tile([128, 512], DT.float32, tag="f2", bufs=1)
            xc = xc[:, 0:QT]
            nc.vector.tensor_scalar(
                xc[:], x_sb[:], -1.0, 1e-12, op0=OP.mult, op1=OP.max
            )
            y0 = junk.tile([128, 256], DT.float32, tag="f3", bufs=1)
            y0 = y0[:, 0:QT]
            nc.scalar.activation(y0[:], xc[:], AF.Sqrt)
            ry = junk.tile([128, 688], DT.float32, tag="f1", bufs=1)
            ry = ry[:, 0:QT]
            nc.vector.reciprocal(ry[:], y0[:])
            t1 = junk.tile([128, QT], DT.float32, tag="s", bufs=2)
            nc.vector.tensor_mul(t1[:], xc[:], ry[:])
            t2 = junk.tile([128, 512], DT.float32, tag="f2", bufs=1)
            t2 = t2[:, 0:QT]
            nc.vector.tensor_add(t2[:], y0[:], t1[:])
            xbf = junk.tile([128, 512], DT.float16, tag="f3", bufs=1)
            xbf = xbf[:, 0:QT]
            nc.vector.tensor_scalar_mul(xbf[:], t2[:], 0.5)

            # ---- MLP (h^T layout: [hid-tile 128, batch 8]) ----
            xg = xbf[:].rearrange("p (b t) -> p t b", t=KT1)
            zero_t = const.tile([128, BPC], DT.float16)
            nc.vector.memset(zero_t[:], 0.0)

            def layer(in_view, w_sb, b_sb, n_kt, n_mt, act_relu, out_dtype):
                # One small psum tile per mt-group: consecutive groups
                # ping-pong the two pool slots, so the relu's PSUM read never
                # serializes against the next group's matmuls (Tile tracks
                # PE-write vs DVE-read conflicts at whole-tile granularity).
                hout = drain.tile([128, n_mt * BPC], out_dtype, tag="h" + str(n_mt))
                for mt in range(n_mt):
                    pt = psum.tile(
                        [128, BPC], DT.float32,
                        tag="psA" if mt % 2 == 0 else "psB", bufs=1,
                    )
                    for kt in range(n_kt):
                        nc.tensor.matmul(
                            pt[:],
                            w_sb[:, ds(kt * n_mt * 128 + mt * 128, 128)],
                            in_view[:, kt, :],
                            start=(kt == 0),
                            stop=(kt == n_kt - 1),
                        )
                    if act_relu:
                        # relu(psum + bias) on VectorE (idle during MLP)
                        nc.vector.scalar_tensor_tensor(
                            hout[:, ds(mt * BPC, BPC)],
                            pt[:],
                            b_sb[:, mt : mt + 1],
                            zero_t[:],
                            op0=OP.add,
                            op1=OP.max,
                        )
                    else:
                        nc.scalar.activation(
                            hout[:, ds(mt * BPC, BPC)],
                            pt[:],
                            AF.Identity,
                            bias=b_sb[:, mt : mt + 1],
                        )
                return hout

            h1 = layer(xg, w0_sb, b0_sb, KT1, MT_H, True, DT.float16)
            h1v = h1[:].rearrange("p (t b) -> p t b", b=BPC)
            h2 = layer(h1v, w1_sb, b1_sb, KT2, MT_H, True, DT.float16)
            h2v = h2[:].rearrange("p (t b) -> p t b", b=BPC)
            h3 = layer(h2v, w2_sb, b2_sb, KT2, MT_H, True, DT.float16)
            h3v = h3[:].rearrange("p (t b) -> p t b", b=BPC)
            h4 = layer(h3v, w3_sb, b3_sb, KT2, MT_O, False, DT.float32)

            for mt in range(MT_O):
                nc.sync.dma_start(outT[mt], h4[:, ds(mt * BPC, BPC)])

    # InstTensorTensorReduce is an extended-inst InstISA subclass; populate
    # its .instr bytes (raw Bass skips this pass -> "ISA wrong length").
    mybir.codegen_inst_isa_subclasses(nc)
    _split_multi_waits(nc)
    return nc


def _split_multi_waits(nc, max_waits=1):
    """neuronx-cc walrus rejects instructions with >1 sync wait; hoist extras
    onto nofuse NOPs just before, on the same engine. Extended-ISA structs
    (InstISA subclasses, e.g. TensorTensorReduce) can't carry ANY sync in
    walrus codegen (setupSyncWait<UNKNOWN_STRUCT>), so for those hoist all
    waits onto a NOP before and all updates onto a NOP after — same-engine
    program order preserves the semantics."""
    ctr = 0
    for f in nc.m.functions:
        for bb in f.blocks:
            new_insts = []
            for ins in bb.instructions:
                limit = 0 if isinstance(ins, mybir.InstISA) else max_waits
                si = getattr(ins, "sync_info", None)
                if si is not None and si.on_wait and len(si.on_wait) > limit:
                    waits = list(si.on_wait)
                    if limit:
                        extra, keep = waits[:-limit], waits[-limit:]
                    else:
                        extra, keep = waits, []
                    step = max(max_waits, 1)
                    for i in range(0, len(extra), step):
                        ctr += 1
                        new_insts.append(
                            mybir.InstNoOp(
                                name=f"waitsplit-{ctr}",
                                engine=ins.engine,
                                sync_info=mybir.SyncInfo(
                                    on_wait=extra[i : i + step], on_update=[]
                                ),
                                bass_nofuse=True,
                            )
                        )
                    si.on_wait = keep
                new_insts.append(ins)
                if isinstance(ins, mybir.InstISA) and si is not None and si.on_update:
                    updates = list(si.on_update)
                    si.on_update = []
                    ctr += 1
                    new_insts.append(
                        mybir.InstNoOp(
                            name=f"updsplit-{ctr}",
                            engine=ins.engine,
                            sync_info=mybir.SyncInfo(on_wait=[], on_update=updates),
                            bass_nofuse=True,
                        )
                    )
            bb.instructions[:] = new_insts


def _prep_inputs(pos, basis, W0, b0, W1, b1, W2, b2, W3, b3):
    pos = np.asarray(pos, dtype=np.float32)
    basis = np.asarray(basis, dtype=np.float32)

    bh, bl = _split_hi_lo(basis)  # [M,3]
    q2 = (basis * basis).sum(-1)
    q2h, q2l = _split_hi_lo(q2)
    ones_m = np.ones(M, np.float32)
    basis_aug = np.zeros((16, M), np.float32)
    basis_aug[0:3] = bh.T
    basis_aug[3:6] = bh.T
    basis_aug[6:9] = bl.T
    basis_aug[9:12] = bl.T
    basis_aug[12] = ones_m
    basis_aug[13] = ones_m
    basis_aug[14] = q2h
    basis_aug[15] = q2l
    # replicate into the 4 PE row-groups (partitions 32g..32g+15)
    basis_rep = np.zeros((128, M), np.float32)
    for g in range(4):
        basis_rep[32 * g : 32 * g + 16] = basis_aug
    basis_rep = basis_rep.astype(BF16)

    def pos_aug_for_core(c):
        p = pos[c * BPC : (c + 1) * BPC].reshape(R, 3)
        a = -2.0 * p
        ah, al = _split_hi_lo(a)
        p2 = (p * p).sum(-1)
        p2h, p2l = _split_hi_lo(p2)
        ones_r = np.ones(R, np.float32)
        pa = np.zeros((16, R), np.float32)
        pa[0:3] = ah.T
        pa[3:6] = al.T
        pa[6:9] = ah.T
        pa[9:12] = al.T
        pa[12] = p2h
        pa[13] = p2l
        pa[14] = ones_r
        pa[15] = ones_r
        pa = -pa  # PSUM accumulates -d2 so the on-chip reduction can be MAX
        pa_rep = np.zeros((128, R), np.float32)
        for g in range(4):
            pa_rep[32 * g : 32 * g + 16] = pa
        return pa_rep.astype(BF16)

    def pack_w(W, n_kt, n_out):
        return (
            np.asarray(W, np.float32)
            .reshape(n_kt, 128, n_out)
            .transpose(1, 0, 2)
            .reshape(128, n_kt * n_out)
            .astype(np.float16)
        )

    common = {
        "basis_aug": basis_rep,
        "w0": pack_w(W0, KT1, HID),
        "w1": pack_w(W1, KT2, HID),
        "w2": pack_w(W2, KT2, HID),
        "w3": pack_w(W3, KT2, OUT),
        "b0t": np.asarray(b0, np.float32).reshape(MT_H, 128).T.copy(),
        "b1t": np.asarray(b1, np.float32).reshape(MT_H, 128).T.copy(),
        "b2t": np.asarray(b2, np.float32).reshape(MT_H, 128).T.copy(),
        "b3t": np.asarray(b3, np.float32).reshape(MT_O, 128).T.copy(),
    }
    in_maps = []
    for c in range(NCORES):
        m = dict(common)
        m["posT_aug"] = pos_aug_for_core(c)
        in_maps.append(m)
    return in_maps


def kernel(pos, basis, W0, b0, W1, b1, W2, b2, W3, b3, _trace=False):
    if "nc" not in _cache:
        _cache["nc"] = _build_program()
    nc = _cache["nc"]
    in_maps = _prep_inputs(pos, basis, W0, b0, W1, b1, W2, b2, W3, b3)
    res = run_bass_kernel_spmd(nc, in_maps, list(range(NCORES)), trace=_trace)
    _cache["last_result"] = res
    out = np.empty((B, OUT), np.float32)
    for c in range(NCORES):
        o = np.asarray(res.results[c]["outT"])  # [MT_O, 128, BPC]
        out[c * BPC : (c + 1) * BPC] = o.transpose(2, 0, 1).reshape(BPC, OUT)
    return out



# revision 45
# speedup vs baseline: 1.0422x; 1.0077x over previous
"""Trainium2 kernel for nn_BpsMlp: KNN min-distance (B=64,N=1024 queries vs
M=4096 basis points) feeding a 4-layer MLP, data-parallel over batch across
8 NeuronCores.

Per core (8 batches = 8192 query rows):
  - distance phase: d2[q,m] accumulated exactly in fp32 PSUM via K=16
    augmented bf16 hi/lo matmuls (catastrophic-cancellation-free), four
    matmuls packed concurrently into the PE via tile_position row-groups.
    PSUM holds -d2 (pos_aug negated on host) so min-reduction is all MAX.
    Egress splits across the only two engines with a PSUM port (ScalarE
    casts 3424 values to fp16 SBUF, VectorE max-pairs the other 672
    against the copy); VectorE 2x-mode folds the fp16 stream one qtile
    deferred, and a fused tensor_scalar max-accum produces
    x_sb[:, t] = -min(d2). (GpSimd is unusable: no PSUM port, and TRN2
    walrus rejects TensorTensor and InstPool on the Pool engine.)
  - x = sqrt(max(d2min, 1e-12)) with one Newton refinement step.
  - MLP in fp16 (weights streamed to SBUF during the distance phase),
    h^T layout [hid-tile 128, batch 8], relu+bias on VectorE.
"""

import sys

sys.path.insert(0, "/opt/trn_rl_repo")

import numpy as np
import ml_dtypes

import concourse.bass as bass
import concourse.mybir as mybir
import concourse.tile as tile
from concourse.bass import ds, ts
from concourse.bass_utils import run_bass_kernel_spmd

BF16 = ml_dtypes.bfloat16
DT = mybir.dt
AF = mybir.ActivationFunctionType
OP = mybir.AluOpType

B, N, M = 64, 1024, 4096
HID, OUT = 2048, 512
NCORES = 8
BPC = B // NCORES            # batches per core
R = BPC * N                  # query rows per core (8192)
QT = R // 128                # q-tiles per core (64)
KAUG = 16                    # augmented contraction dim
MT_H = HID // 128            # hid tiles (16)
KT1 = N // 128               # L1 k-tiles (8)
KT2 = HID // 128             # L2/L3/L4 k-tiles (16)
MT_O = OUT // 128            # out tiles (4)

_cache = {}


def _split_hi_lo(v):
    vh = v.astype(BF16).astype(np.float32)
    vl = (v - vh).astype(BF16).astype(np.float32)
    return vh, vl


def _build_program():
    nc = bass.Bass()

    posT = nc.declare_dram_parameter("posT_aug", [128, R], DT.bfloat16, isOutput=False)
    basisA = nc.declare_dram_parameter("basis_aug", [128, M], DT.bfloat16, isOutput=False)
    w0 = nc.declare_dram_parameter("w0", [128, KT1 * HID], DT.float16, isOutput=False)
    w1 = nc.declare_dram_parameter("w1", [128, KT2 * HID], DT.float16, isOutput=False)
    w2 = nc.declare_dram_parameter("w2", [128, KT2 * HID], DT.float16, isOutput=False)
    w3 = nc.declare_dram_parameter("w3", [128, KT2 * OUT], DT.float16, isOutput=False)
    b0d = nc.declare_dram_parameter("b0t", [128, MT_H], DT.float32, isOutput=False)
    b1d = nc.declare_dram_parameter("b1t", [128, MT_H], DT.float32, isOutput=False)
    b2d = nc.declare_dram_parameter("b2t", [128, MT_H], DT.float32, isOutput=False)
    b3d = nc.declare_dram_parameter("b3t", [128, MT_O], DT.float32, isOutput=False)
    outT = nc.declare_dram_parameter("outT", [MT_O, 128, BPC], DT.float32, isOutput=True)

    with tile.TileContext(nc) as tc:
        with (
            tc.tile_pool(name="const", bufs=1) as const,
            tc.tile_pool(name="psum", bufs=2, space="PSUM") as psum,
            tc.tile_pool(name="cpp", bufs=2) as cpp,
            tc.tile_pool(name="drain", bufs=2) as drain,
            tc.tile_pool(name="junk", bufs=1) as junk,
            tc.tile_pool(name="posc", bufs=2) as posc,
        ):
            basis_sb = const.tile([128, M], DT.bfloat16)

            w0_sb = const.tile([128, KT1 * HID], DT.float16)
            w1_sb = const.tile([128, KT2 * HID], DT.float16)
            w2_sb = const.tile([128, KT2 * HID], DT.float16)
            w3_sb = const.tile([128, KT2 * OUT], DT.float16)
            b0_sb = const.tile([128, MT_H], DT.float32)
            b1_sb = const.tile([128, MT_H], DT.float32)
            b2_sb = const.tile([128, MT_H], DT.float32)
            b3_sb = const.tile([128, MT_O], DT.float32)

            x_sb = const.tile([128, QT], DT.float32)

            # touch Sqrt now so its 1.3us activation-table load happens under
            # the startup DMAs instead of between the distance and MLP phases
            warm = const.tile([128, 1], DT.float32)
            nc.vector.memset(warm[:], 1.0)
            nc.scalar.activation(warm[:], warm[:], AF.Sqrt)

            # ---- distance phase ----
            # pos/basis augmented rows replicated into 4 PE row-groups so the
            # four K=16 matmuls per unit run concurrently (tile_position).
            # Every d2 value crosses exactly one of the two PSUM read paths
            # (ScalarE 1.2 GHz / VectorE 0.96 GHz), which is the structural
            # drain floor; PSUM has no GpSimd port and one DVE read port.
            # MLP weight DMAs are spread across the blocks so the pos-chunk
            # prefetches never sit behind a deep weight backlog.
            wdmas = []
            for j in range(KT1):
                wdmas.append((w0_sb[:, ts(j, HID)], w0[:, ts(j, HID)]))
            for j in range(KT2):
                wdmas.append((w1_sb[:, ts(j, HID)], w1[:, ts(j, HID)]))
                wdmas.append((w2_sb[:, ts(j, HID)], w2[:, ts(j, HID)]))
                wdmas.append((w3_sb[:, ts(j, OUT)], w3[:, ts(j, OUT)]))
            # biases are 64 B each — issue up front, they can't clog a queue
            nc.sync.dma_start(b0_sb[:], b0d[:])
            nc.sync.dma_start(b1_sb[:], b1d[:])
            nc.sync.dma_start(b2_sb[:], b2d[:])
            nc.sync.dma_start(b3_sb[:], b3d[:])
            wd_i = 0

            pos_tiles = {}

            def fold_block(t, cp_, s_):
                # Deferred fold of qtile t (issued during qtile t+1 so DVE
                # never stalls on ScalarE's second copy): 2x-mode max-folds
                # of the 2752 leftover copies and the 672 paired maxima,
                # then one fused tensor_scalar max-accum into x_sb[:, t].
                f2 = junk.tile([128, 1024], DT.float16, tag="f2", bufs=1)
                nc.vector.tensor_tensor(
                    f2[:, 688:1024], s_[:, 0:336], s_[:, 336:672], op=OP.max
                )
                f1 = junk.tile([128, 1376], DT.float16, tag="f1", bufs=1)
                nc.vector.tensor_tensor(
                    f1[:], cp_[:, 672:2048], cp_[:, 2048:3424], op=OP.max
                )
                nc.vector.tensor_tensor(f2[:, 0:688], f1[:, 0:688], f1[:, 688:1376], op=OP.max)
                f3 = junk.tile([128, 512], DT.float16, tag="f3", bufs=1)
                nc.vector.tensor_tensor(f3[:], f2[:, 0:512], f2[:, 512:1024], op=OP.max)
                fo = junk.tile([128, 1376], DT.float16, tag="f1", bufs=1)
                nc.vector.tensor_scalar(
                    fo[:, 0:512], f3[:], 1.0, None,
                    op0=OP.mult, op1=OP.max, accum_out=x_sb[:, t : t + 1],
                )

            def issue_chunk(c):
                pc_ = posc.tile([128, 128], DT.bfloat16, tag="posc")
                nc.sync.dma_start(pc_[:, 0:64], posT[:, ds(c * 128, 64)])
                nc.sync.dma_start(pc_[:, 64:128], posT[:, ds(c * 128 + 64, 64)])
                pos_tiles[c] = pc_

            # startup critical path: qtile 0 needs pos chunk 0 and basis cols
            # [0,512). Issue those first (the sync sequencer serializes issues
            # at ~600ns each), the 512-col slice as 4 parallel 128-col DMAs,
            # then the rest of basis.
            issue_chunk(0)
            for j in range(4):
                nc.sync.dma_start(basis_sb[:, ts(j, 128)], basisA[:, ts(j, 128)])
            for j in range(1, 8):
                nc.sync.dma_start(basis_sb[:, ts(j, M // 8)], basisA[:, ts(j, M // 8)])
            pending = None
            for t in range(QT):
                if t + 1 < QT:
                    issue_chunk(t + 1)
                if t >= 8 and wd_i < len(wdmas):
                    # exactly one weight DMA per qtile, none during startup:
                    # a 512 KB weight chunk queued ahead of a pos-chunk DMA
                    # stalls the PE ~7us, and block-issuing weights loads 8
                    # consecutive queues at once so the round-robin rotation
                    # wraps onto a still-draining queue. One per qtile gives
                    # each queue ~56us to drain its 23us transfer.
                    dst, src = wdmas[wd_i]
                    nc.sync.dma_start(dst, src)
                    wd_i += 1
                pos_chunk = pos_tiles[t]
                # PSUM holds -d2 (pos_aug negated on the host) so every
                # reduction is a MAX — required for GpSimd's pool (max-only).
                # tile A: m in [0,2048). Drained by ScalarE alone (fp16 cast
                # to SBUF), freeing its banks early for the next qtile.
                ptA = psum.tile([128, 2048], DT.float32, tag="psA", bufs=1)
                for j in range(4):
                    nc.tensor.matmul(
                        ptA[:, ts(j, 512)],
                        pos_chunk[32 * j : 32 * j + KAUG, 0:128],
                        basis_sb[32 * j : 32 * j + KAUG, ts(j, 512)],
                        tile_position=(32 * j, 0),
                    )
                cp = cpp.tile([128, 3424], DT.float16, tag="cp")
                nc.scalar.copy(cp[:, 0:2048], ptA[:])
                # tile B: m in [2048,4096).
                ptB = psum.tile([128, 2048], DT.float32, tag="psB", bufs=1)
                for j in range(4):
                    nc.tensor.matmul(
                        ptB[:, ts(j, 512)],
                        pos_chunk[32 * j : 32 * j + KAUG, 0:128],
                        basis_sb[32 * j : 32 * j + KAUG, ds(2048 + j * 512, 512)],
                        tile_position=(32 * j, 0),
                    )
                # Balanced 2-engine drain (PSUM egress is the wall: only DVE
                # ~1.04ns/el and Act ~0.83ns/el have PSUM ports; GpSimd has
                # no PSUM port and TRN2 walrus rejects every GpSimd free-axis
                # compute op): Act egresses A (2048) + B[0:1376]; DVE
                # egresses B[1376:] paired against A's copy (one max level
                # for free); the fp16 fold runs on DVE in 2x mode, deferred
                # one qtile (fold_block).
                nc.scalar.copy(cp[:, 2048:3424], ptB[:, 0:1376])
                s = junk.tile([128, 672], DT.float16, tag="s", bufs=2)
                nc.vector.tensor_tensor(s[:], ptB[:, 1376:2048], cp[:, 0:672], op=OP.max)
                if pending is not None:
                    fold_block(*pending)
                pending = (t, cp, s)
            fold_block(*pending)

            # ---- x = sqrt(max(d2,1e-12)), one Newton step ----
            # x_sb holds -min(d2); negate while clamping.
            # temps live in recycled junk-pool slots (SBUF is tight).
            xc = junk.tile([128, 512], DT.float32, tag="f2", bufs=1)
            xc = xc[:, 0:QT]
            nc.vector.tensor_scalar(
                xc[:], x_sb[:], -1.0, 1e-12, op0=OP.mult, op1=OP.max
            )
            y0 = junk.tile([128, 256], DT.float32, tag="f3", bufs=1)
            y0 = y0[:, 0:QT]
            nc.scalar.activation(y0[:], xc[:], AF.Sqrt)
            xbf = junk.tile([128, 512], DT.float16, tag="f2", bufs=1)
            xbf = xbf[:, 0:QT]
            nc.vector.tensor_scalar_mul(xbf[:], y0[:], 1.0)

            # ---- MLP (h^T layout: [hid-tile 128, batch 8]) ----
            xg = xbf[:].rearrange("p (b t) -> p t b", t=KT1)
            zero_t = const.tile([128, BPC], DT.float16)
            nc.vector.memset(zero_t[:], 0.0)

            def layer(in_view, w_sb, b_sb, n_kt, n_mt, act_relu, out_dtype):
                # One small psum tile per mt-group: consecutive groups
                # ping-pong the two pool slots, so the relu's PSUM read never
                # serializes against the next group's matmuls (Tile tracks
                # PE-write vs DVE-read conflicts at whole-tile granularity).
                hout = drain.tile([128, n_mt * BPC], out_dtype, tag="h" + str(n_mt))
                for mt in range(n_mt):
                    pt = psum.tile(
                        [128, BPC], DT.float32,
                        tag="psA" if mt % 2 == 0 else "psB", bufs=1,
                    )
                    for kt in range(n_kt):
                        nc.tensor.matmul(
                            pt[:],
                            w_sb[:, ds(kt * n_mt * 128 + mt * 128, 128)],
                            in_view[:, kt, :],
                            start=(kt == 0),
                            stop=(kt == n_kt - 1),
                        )
                    if act_relu:
                        # relu(psum + bias) on VectorE (idle during MLP)
                        nc.vector.scalar_tensor_tensor(
                            hout[:, ds(mt * BPC, BPC)],
                            pt[:],
                            b_sb[:, mt : mt + 1],
                            zero_t[:],
                            op0=OP.add,
                            op1=OP.max,
                        )
                    else:
                        nc.scalar.activation(
                            hout[:, ds(mt * BPC, BPC)],
                            pt[:],
                            AF.Identity,
                            bias=b_sb[:, mt : mt + 1],
                        )
                return hout

            h1 = layer(xg, w0_sb, b0_sb, KT1, MT_H, True, DT.float16)
            h1v = h1[:].rearrange("p (t b) -> p t b", b=BPC)
            h2 = layer(h1v, w1_sb, b1_sb, KT2, MT_H, True, DT.float16)
            h2v = h2[:].rearrange("p (t b) -> p t b", b=BPC)
            h3 = layer(h2v, w2_sb, b2_sb, KT2, MT_H, True, DT.float16)
            h3v = h3[:].rearrange("p (t b) -> p t b", b=BPC)
            h4 = layer(h3v, w3_sb, b3_sb, KT2, MT_O, False, DT.float32)

            for mt in range(MT_O):
                nc.sync.dma_start(outT[mt], h4[:, ds(mt * BPC, BPC)])

    # InstTensorTensorReduce is an extended-inst InstISA subclass; populate
    # its .instr bytes (raw Bass skips this pass -> "ISA wrong length").
    mybir.codegen_inst_isa_subclasses(nc)
    _split_multi_waits(nc)
    return nc


def _split_multi_waits(nc, max_waits=1):
    """neuronx-cc walrus rejects instructions with >1 sync wait; hoist extras
    onto nofuse NOPs just before, on the same engine. Extended-ISA structs
    (InstISA subclasses, e.g. TensorTensorReduce) can't carry ANY sync in
    walrus codegen (setupSyncWait<UNKNOWN_STRUCT>), so for those hoist all
    waits onto a NOP before and all updates onto a NOP after — same-engine
    program order preserves the semantics."""
    ctr = 0
    for f in nc.m.functions:
        for bb in f.blocks:
            new_insts = []
            for ins in bb.instructions:
                limit = 0 if isinstance(ins, mybir.InstISA) else max_waits
                si = getattr(ins, "sync_info", None)
                if si is not None and si.on_wait and len(si.on_wait) > limit:
                    waits = list(si.on_wait)
                    if limit:
                        extra, keep = waits[:-limit], waits[-limit:]
                    else:
                        extra, keep = waits, []
                    step = max(max_waits, 1)
                    for i in range(0, len(extra), step):
                        ctr += 1
                        new_insts.append(
                            mybir.InstNoOp(
                                name=f"waitsplit-{ctr}",
                                engine=ins.engine,
                                sync_info=mybir.SyncInfo(
                                    on_wait=extra[i : i + step], on_update=[]
                                ),
                                bass_nofuse=True,
                            )
                        )
                    si.on_wait = keep
                new_insts.append(ins)
                if isinstance(ins, mybir.InstISA) and si is not None and si.on_update:
                    updates = list(si.on_update)
                    si.on_update = []
                    ctr += 1
                    new_insts.append(
                        mybir.InstNoOp(
                            name=f"updsplit-{ctr}",
                            engine=ins.engine,
                            sync_info=mybir.SyncInfo(on_wait=[], on_update=updates),
                            bass_nofuse=True,
                        )
                    )
            bb.instructions[:] = new_insts


def _prep_inputs(pos, basis, W0, b0, W1, b1, W2, b2, W3, b3):
    pos = np.asarray(pos, dtype=np.float32)
    basis = np.asarray(basis, dtype=np.float32)

    bh, bl = _split_hi_lo(basis)  # [M,3]
    q2 = (basis * basis).sum(-1)
    q2h, q2l = _split_hi_lo(q2)
    ones_m = np.ones(M, np.float32)
    basis_aug = np.zeros((16, M), np.float32)
    basis_aug[0:3] = bh.T
    basis_aug[3:6] = bh.T
    basis_aug[6:9] = bl.T
    basis_aug[9:12] = bl.T
    basis_aug[12] = ones_m
    basis_aug[13] = ones_m
    basis_aug[14] = q2h
    basis_aug[15] = q2l
    # replicate into the 4 PE row-groups (partitions 32g..32g+15)
    basis_rep = np.zeros((128, M), np.float32)
    for g in range(4):
        basis_rep[32 * g : 32 * g + 16] = basis_aug
    basis_rep = basis_rep.astype(BF16)

    def pos_aug_for_core(c):
        p = pos[c * BPC : (c + 1) * BPC].reshape(R, 3)
        a = -2.0 * p
        ah, al = _split_hi_lo(a)
        p2 = (p * p).sum(-1)
        p2h, p2l = _split_hi_lo(p2)
        ones_r = np.ones(R, np.float32)
        pa = np.zeros((16, R), np.float32)
        pa[0:3] = ah.T
        pa[3:6] = al.T
        pa[6:9] = ah.T
        pa[9:12] = al.T
        pa[12] = p2h
        pa[13] = p2l
        pa[14] = ones_r
        pa[15] = ones_r
        pa = -pa  # PSUM accumulates -d2 so the on-chip reduction can be MAX
        pa_rep = np.zeros((128, R), np.float32)
        for g in range(4):
            pa_rep[32 * g : 32 * g + 16] = pa
        return pa_rep.astype(BF16)

    def pack_w(W, n_kt, n_out):
        return (
            np.asarray(W, np.float32)
            .reshape(n_kt, 128, n_out)
            .transpose(1, 0, 2)
            .reshape(128, n_kt * n_out)
            .astype(np.float16)
        )

    common = {
        "basis_aug": basis_rep,
        "w0": pack_w(W0, KT1, HID),
        "w1": pack_w(W1, KT2, HID),
        "w2": pack_w(W2, KT2, HID),
        "w3": pack_w(W3, KT2, OUT),
        "b0t": np.asarray(b0, np.float32).reshape(MT_H, 128).T.copy(),
        "b1t": np.asarray(b1, np.float32).reshape(MT_H, 128).T.copy(),
        "b2t": np.asarray(b2, np.float32).reshape(MT_H, 128).T.copy(),
        "b3t": np.asarray(b3, np.float32).reshape(MT_O, 128).T.copy(),
    }
    in_maps = []
    for c in range(NCORES):
        m = dict(common)
        m["posT_aug"] = pos_aug_for_core(c)
        in_maps.append(m)
    return in_maps


def kernel(pos, basis, W0, b0, W1, b1, W2, b2, W3, b3, _trace=False):
    if "nc" not in _cache:
        _cache["nc"] = _build_program()
    nc = _cache["nc"]
    in_maps = _prep_inputs(pos, basis, W0, b0, W1, b1, W2, b2, W3, b3)
    res = run_bass_kernel_spmd(nc, in_maps, list(range(NCORES)), trace=_trace)
    _cache["last_result"] = res
    out = np.empty((B, OUT), np.float32)
    for c in range(NCORES):
        o = np.asarray(res.results[c]["outT"])  # [MT_O, 128, BPC]
        out[c * BPC : (c + 1) * BPC] = o.transpose(2, 0, 1).reshape(BPC, OUT)
    return out



# revision 46
# speedup vs baseline: 1.0442x; 1.0020x over previous
"""Trainium2 kernel for nn_BpsMlp: KNN min-distance (B=64,N=1024 queries vs
M=4096 basis points) feeding a 4-layer MLP, data-parallel over batch across
8 NeuronCores.

Per core (8 batches = 8192 query rows):
  - distance phase: d2[q,m] accumulated exactly in fp32 PSUM via K=16
    augmented bf16 hi/lo matmuls (catastrophic-cancellation-free), four
    matmuls packed concurrently into the PE via tile_position row-groups.
    PSUM holds -d2 (pos_aug negated on host) so min-reduction is all MAX.
    Egress splits across the only two engines with a PSUM port (ScalarE
    casts 3424 values to fp16 SBUF, VectorE max-pairs the other 672
    against the copy); VectorE 2x-mode folds the fp16 stream one qtile
    deferred, and a fused tensor_scalar max-accum produces
    x_sb[:, t] = -min(d2). (GpSimd is unusable: no PSUM port, and TRN2
    walrus rejects TensorTensor and InstPool on the Pool engine.)
  - x = sqrt(max(d2min, 1e-12)) with one Newton refinement step.
  - MLP in fp16 (weights streamed to SBUF during the distance phase),
    h^T layout [hid-tile 128, batch 8], relu+bias on VectorE.
"""

import sys

sys.path.insert(0, "/opt/trn_rl_repo")

import numpy as np
import ml_dtypes

import concourse.bass as bass
import concourse.mybir as mybir
import concourse.tile as tile
from concourse.bass import ds, ts
from concourse.bass_utils import run_bass_kernel_spmd

BF16 = ml_dtypes.bfloat16
DT = mybir.dt
AF = mybir.ActivationFunctionType
OP = mybir.AluOpType

B, N, M = 64, 1024, 4096
HID, OUT = 2048, 512
NCORES = 8
BPC = B // NCORES            # batches per core
R = BPC * N                  # query rows per core (8192)
QT = R // 128                # q-tiles per core (64)
KAUG = 16                    # augmented contraction dim
MT_H = HID // 128            # hid tiles (16)
KT1 = N // 128               # L1 k-tiles (8)
KT2 = HID // 128             # L2/L3/L4 k-tiles (16)
MT_O = OUT // 128            # out tiles (4)

_cache = {}


def _split_hi_lo(v):
    vh = v.astype(BF16).astype(np.float32)
    vl = (v - vh).astype(BF16).astype(np.float32)
    return vh, vl


def _build_program():
    nc = bass.Bass()

    posT = nc.declare_dram_parameter("posT_aug", [128, R], DT.bfloat16, isOutput=False)
    basisA = nc.declare_dram_parameter("basis_aug", [128, M], DT.bfloat16, isOutput=False)
    w0 = nc.declare_dram_parameter("w0", [128, KT1 * HID], DT.float16, isOutput=False)
    w1 = nc.declare_dram_parameter("w1", [128, KT2 * HID], DT.float16, isOutput=False)
    w2 = nc.declare_dram_parameter("w2", [128, KT2 * HID], DT.float16, isOutput=False)
    w3 = nc.declare_dram_parameter("w3", [128, KT2 * OUT], DT.float16, isOutput=False)
    b0d = nc.declare_dram_parameter("b0t", [128, MT_H], DT.float32, isOutput=False)
    b1d = nc.declare_dram_parameter("b1t", [128, MT_H], DT.float32, isOutput=False)
    b2d = nc.declare_dram_parameter("b2t", [128, MT_H], DT.float32, isOutput=False)
    b3d = nc.declare_dram_parameter("b3t", [128, MT_O], DT.float32, isOutput=False)
    outT = nc.declare_dram_parameter("outT", [MT_O, 128, BPC], DT.float32, isOutput=True)

    with tile.TileContext(nc) as tc:
        with (
            tc.tile_pool(name="const", bufs=1) as const,
            tc.tile_pool(name="psum", bufs=2, space="PSUM") as psum,
            tc.tile_pool(name="cpp", bufs=2) as cpp,
            tc.tile_pool(name="drain", bufs=2) as drain,
            tc.tile_pool(name="junk", bufs=1) as junk,
            tc.tile_pool(name="posc", bufs=2) as posc,
        ):
            basis_sb = const.tile([128, M], DT.bfloat16)

            w0_sb = const.tile([128, KT1 * HID], DT.float16)
            w1_sb = const.tile([128, KT2 * HID], DT.float16)
            w2_sb = const.tile([128, KT2 * HID], DT.float16)
            w3_sb = const.tile([128, KT2 * OUT], DT.float16)
            b0_sb = const.tile([128, MT_H], DT.float32)
            b1_sb = const.tile([128, MT_H], DT.float32)
            b2_sb = const.tile([128, MT_H], DT.float32)
            b3_sb = const.tile([128, MT_O], DT.float32)

            x_sb = const.tile([128, QT], DT.float32)

            # touch Sqrt now so its 1.3us activation-table load happens under
            # the startup DMAs instead of between the distance and MLP phases
            warm = const.tile([128, 1], DT.float32)
            nc.vector.memset(warm[:], 1.0)
            nc.scalar.activation(warm[:], warm[:], AF.Sqrt)

            # ---- distance phase ----
            # pos/basis augmented rows replicated into 4 PE row-groups so the
            # four K=16 matmuls per unit run concurrently (tile_position).
            # Every d2 value crosses exactly one of the two PSUM read paths
            # (ScalarE 1.2 GHz / VectorE 0.96 GHz), which is the structural
            # drain floor; PSUM has no GpSimd port and one DVE read port.
            # MLP weight DMAs are spread across the blocks so the pos-chunk
            # prefetches never sit behind a deep weight backlog.
            wdmas = []
            for j in range(KT1):
                wdmas.append((w0_sb[:, ts(j, HID)], w0[:, ts(j, HID)]))
            for j in range(KT2):
                wdmas.append((w1_sb[:, ts(j, HID)], w1[:, ts(j, HID)]))
                wdmas.append((w2_sb[:, ts(j, HID)], w2[:, ts(j, HID)]))
                wdmas.append((w3_sb[:, ts(j, OUT)], w3[:, ts(j, OUT)]))
            wd_i = 0

            pos_tiles = {}

            def fold_block(t, cp_, s_):
                # Deferred fold of qtile t (issued during qtile t+1 so DVE
                # never stalls on ScalarE's second copy): 2x-mode max-folds
                # of the 2752 leftover copies and the 672 paired maxima,
                # then one fused tensor_scalar max-accum into x_sb[:, t].
                f2 = junk.tile([128, 1024], DT.float16, tag="f2", bufs=1)
                nc.vector.tensor_tensor(
                    f2[:, 688:1024], s_[:, 0:336], s_[:, 336:672], op=OP.max
                )
                f1 = junk.tile([128, 1376], DT.float16, tag="f1", bufs=1)
                nc.vector.tensor_tensor(
                    f1[:], cp_[:, 672:2048], cp_[:, 2048:3424], op=OP.max
                )
                nc.vector.tensor_tensor(f2[:, 0:688], f1[:, 0:688], f1[:, 688:1376], op=OP.max)
                f3 = junk.tile([128, 512], DT.float16, tag="f3", bufs=1)
                nc.vector.tensor_tensor(f3[:], f2[:, 0:512], f2[:, 512:1024], op=OP.max)
                fo = junk.tile([128, 1376], DT.float16, tag="f1", bufs=1)
                nc.vector.tensor_scalar(
                    fo[:, 0:512], f3[:], 1.0, None,
                    op0=OP.mult, op1=OP.max, accum_out=x_sb[:, t : t + 1],
                )

            def issue_chunk(c):
                pc_ = posc.tile([128, 128], DT.bfloat16, tag="posc")
                nc.sync.dma_start(pc_[:, 0:64], posT[:, ds(c * 128, 64)])
                nc.sync.dma_start(pc_[:, 64:128], posT[:, ds(c * 128 + 64, 64)])
                pos_tiles[c] = pc_

            # startup critical path: qtile 0 needs pos chunk 0 and basis cols
            # [0,512). Issue those first (the sync sequencer serializes issues
            # at ~600ns each), the 512-col slice as 4 parallel 128-col DMAs,
            # then the rest of basis.
            issue_chunk(0)
            for j in range(4):
                nc.sync.dma_start(basis_sb[:, ts(j, 128)], basisA[:, ts(j, 128)])
            for j in range(1, 8):
                nc.sync.dma_start(basis_sb[:, ts(j, M // 8)], basisA[:, ts(j, M // 8)])
            # biases are 64 B each and not needed until the MLP — keep their
            # 4 sequencer issues (~600ns each) off the startup critical path
            nc.sync.dma_start(b0_sb[:], b0d[:])
            nc.sync.dma_start(b1_sb[:], b1d[:])
            nc.sync.dma_start(b2_sb[:], b2d[:])
            nc.sync.dma_start(b3_sb[:], b3d[:])
            pending = None
            for t in range(QT):
                if t + 1 < QT:
                    issue_chunk(t + 1)
                if t >= 8 and wd_i < len(wdmas):
                    # exactly one weight DMA per qtile, none during startup:
                    # a 512 KB weight chunk queued ahead of a pos-chunk DMA
                    # stalls the PE ~7us, and block-issuing weights loads 8
                    # consecutive queues at once so the round-robin rotation
                    # wraps onto a still-draining queue. One per qtile gives
                    # each queue ~56us to drain its 23us transfer.
                    dst, src = wdmas[wd_i]
                    nc.sync.dma_start(dst, src)
                    wd_i += 1
                pos_chunk = pos_tiles[t]
                # PSUM holds -d2 (pos_aug negated on the host) so every
                # reduction is a MAX — required for GpSimd's pool (max-only).
                # tile A: m in [0,2048). Drained by ScalarE alone (fp16 cast
                # to SBUF), freeing its banks early for the next qtile.
                ptA = psum.tile([128, 2048], DT.float32, tag="psA", bufs=1)
                for j in range(4):
                    nc.tensor.matmul(
                        ptA[:, ts(j, 512)],
                        pos_chunk[32 * j : 32 * j + KAUG, 0:128],
                        basis_sb[32 * j : 32 * j + KAUG, ts(j, 512)],
                        tile_position=(32 * j, 0),
                    )
                cp = cpp.tile([128, 3424], DT.float16, tag="cp")
                nc.scalar.copy(cp[:, 0:2048], ptA[:])
                # tile B: m in [2048,4096).
                ptB = psum.tile([128, 2048], DT.float32, tag="psB", bufs=1)
                for j in range(4):
                    nc.tensor.matmul(
                        ptB[:, ts(j, 512)],
                        pos_chunk[32 * j : 32 * j + KAUG, 0:128],
                        basis_sb[32 * j : 32 * j + KAUG, ds(2048 + j * 512, 512)],
                        tile_position=(32 * j, 0),
                    )
                # Balanced 2-engine drain (PSUM egress is the wall: only DVE
                # ~1.04ns/el and Act ~0.83ns/el have PSUM ports; GpSimd has
                # no PSUM port and TRN2 walrus rejects every GpSimd free-axis
                # compute op): Act egresses A (2048) + B[0:1376]; DVE
                # egresses B[1376:] paired against A's copy (one max level
                # for free); the fp16 fold runs on DVE in 2x mode, deferred
                # one qtile (fold_block).
                nc.scalar.copy(cp[:, 2048:3424], ptB[:, 0:1376])
                s = junk.tile([128, 672], DT.float16, tag="s", bufs=2)
                nc.vector.tensor_tensor(s[:], ptB[:, 1376:2048], cp[:, 0:672], op=OP.max)
                if pending is not None:
                    fold_block(*pending)
                pending = (t, cp, s)
            fold_block(*pending)

            # ---- x = sqrt(-(-min d2)) ----
            # x_sb holds -min(d2) <= 0; the Sqrt activation negates via
            # scale=-1 (no clamp needed: the Newton reciprocal it guarded is
            # gone, and table-Sqrt(0)=0 is fine).
            y0 = junk.tile([128, 256], DT.float32, tag="f3", bufs=1)
            y0 = y0[:, 0:QT]
            nc.scalar.activation(y0[:], x_sb[:], AF.Sqrt, scale=-1.0)
            xbf = junk.tile([128, 512], DT.float16, tag="f2", bufs=1)
            xbf = xbf[:, 0:QT]
            nc.vector.tensor_scalar_mul(xbf[:], y0[:], 1.0)

            # ---- MLP (h^T layout: [hid-tile 128, batch 8]) ----
            xg = xbf[:].rearrange("p (b t) -> p t b", t=KT1)
            zero_t = const.tile([128, BPC], DT.float16)
            nc.vector.memset(zero_t[:], 0.0)

            def layer(in_view, w_sb, b_sb, n_kt, n_mt, act_relu, out_dtype):
                # One small psum tile per mt-group: consecutive groups
                # ping-pong the two pool slots, so the relu's PSUM read never
                # serializes against the next group's matmuls (Tile tracks
                # PE-write vs DVE-read conflicts at whole-tile granularity).
                hout = drain.tile([128, n_mt * BPC], out_dtype, tag="h" + str(n_mt))
                for mt in range(n_mt):
                    pt = psum.tile(
                        [128, BPC], DT.float32,
                        tag="psA" if mt % 2 == 0 else "psB", bufs=1,
                    )
                    for kt in range(n_kt):
                        nc.tensor.matmul(
                            pt[:],
                            w_sb[:, ds(kt * n_mt * 128 + mt * 128, 128)],
                            in_view[:, kt, :],
                            start=(kt == 0),
                            stop=(kt == n_kt - 1),
                        )
                    if act_relu:
                        # relu(psum + bias) on VectorE (idle during MLP)
                        nc.vector.scalar_tensor_tensor(
                            hout[:, ds(mt * BPC, BPC)],
                            pt[:],
                            b_sb[:, mt : mt + 1],
                            zero_t[:],
                            op0=OP.add,
                            op1=OP.max,
                        )
                    else:
                        nc.scalar.activation(
                            hout[:, ds(mt * BPC, BPC)],
                            pt[:],
                            AF.Identity,
                            bias=b_sb[:, mt : mt + 1],
                        )
                return hout

            h1 = layer(xg, w0_sb, b0_sb, KT1, MT_H, True, DT.float16)
            h1v = h1[:].rearrange("p (t b) -> p t b", b=BPC)
            h2 = layer(h1v, w1_sb, b1_sb, KT2, MT_H, True, DT.float16)
            h2v = h2[:].rearrange("p (t b) -> p t b", b=BPC)
            h3 = layer(h2v, w2_sb, b2_sb, KT2, MT_H, True, DT.float16)
            h3v = h3[:].rearrange("p (t b) -> p t b", b=BPC)
            h4 = layer(h3v, w3_sb, b3_sb, KT2, MT_O, False, DT.float32)

            for mt in range(MT_O):
                nc.sync.dma_start(outT[mt], h4[:, ds(mt * BPC, BPC)])

    # InstTensorTensorReduce is an extended-inst InstISA subclass; populate
    # its .instr bytes (raw Bass skips this pass -> "ISA wrong length").
    mybir.codegen_inst_isa_subclasses(nc)
    _split_multi_waits(nc)
    return nc


def _split_multi_waits(nc, max_waits=1):
    """neuronx-cc walrus rejects instructions with >1 sync wait; hoist extras
    onto nofuse NOPs just before, on the same engine. Extended-ISA structs
    (InstISA subclasses, e.g. TensorTensorReduce) can't carry ANY sync in
    walrus codegen (setupSyncWait<UNKNOWN_STRUCT>), so for those hoist all
    waits onto a NOP before and all updates onto a NOP after — same-engine
    program order preserves the semantics."""
    ctr = 0
    for f in nc.m.functions:
        for bb in f.blocks:
            new_insts = []
            for ins in bb.instructions:
                limit = 0 if isinstance(ins, mybir.InstISA) else max_waits
                si = getattr(ins, "sync_info", None)
                if si is not None and si.on_wait and len(si.on_wait) > limit:
                    waits = list(si.on_wait)
                    if limit:
                        extra, keep = waits[:-limit], waits[-limit:]
                    else:
                        extra, keep = waits, []
                    step = max(max_waits, 1)
                    for i in range(0, len(extra), step):
                        ctr += 1
                        new_insts.append(
                            mybir.InstNoOp(
                                name=f"waitsplit-{ctr}",
                                engine=ins.engine,
                                sync_info=mybir.SyncInfo(
                                    on_wait=extra[i : i + step], on_update=[]
                                ),
                                bass_nofuse=True,
                            )
                        )
                    si.on_wait = keep
                new_insts.append(ins)
                if isinstance(ins, mybir.InstISA) and si is not None and si.on_update:
                    updates = list(si.on_update)
                    si.on_update = []
                    ctr += 1
                    new_insts.append(
                        mybir.InstNoOp(
                            name=f"updsplit-{ctr}",
                            engine=ins.engine,
                            sync_info=mybir.SyncInfo(on_wait=[], on_update=updates),
                            bass_nofuse=True,
                        )
                    )
            bb.instructions[:] = new_insts


def _prep_inputs(pos, basis, W0, b0, W1, b1, W2, b2, W3, b3):
    pos = np.asarray(pos, dtype=np.float32)
    basis = np.asarray(basis, dtype=np.float32)

    bh, bl = _split_hi_lo(basis)  # [M,3]
    q2 = (basis * basis).sum(-1)
    q2h, q2l = _split_hi_lo(q2)
    ones_m = np.ones(M, np.float32)
    basis_aug = np.zeros((16, M), np.float32)
    basis_aug[0:3] = bh.T
    basis_aug[3:6] = bh.T
    basis_aug[6:9] = bl.T
    basis_aug[9:12] = bl.T
    basis_aug[12] = ones_m
    basis_aug[13] = ones_m
    basis_aug[14] = q2h
    basis_aug[15] = q2l
    # replicate into the 4 PE row-groups (partitions 32g..32g+15)
    basis_rep = np.zeros((128, M), np.float32)
    for g in range(4):
        basis_rep[32 * g : 32 * g + 16] = basis_aug
    basis_rep = basis_rep.astype(BF16)

    def pos_aug_for_core(c):
        p = pos[c * BPC : (c + 1) * BPC].reshape(R, 3)
        a = -2.0 * p
        ah, al = _split_hi_lo(a)
        p2 = (p * p).sum(-1)
        p2h, p2l = _split_hi_lo(p2)
        ones_r = np.ones(R, np.float32)
        pa = np.zeros((16, R), np.float32)
        pa[0:3] = ah.T
        pa[3:6] = al.T
        pa[6:9] = ah.T
        pa[9:12] = al.T
        pa[12] = p2h
        pa[13] = p2l
        pa[14] = ones_r
        pa[15] = ones_r
        pa = -pa  # PSUM accumulates -d2 so the on-chip reduction can be MAX
        pa_rep = np.zeros((128, R), np.float32)
        for g in range(4):
            pa_rep[32 * g : 32 * g + 16] = pa
        return pa_rep.astype(BF16)

    def pack_w(W, n_kt, n_out):
        return (
            np.asarray(W, np.float32)
            .reshape(n_kt, 128, n_out)
            .transpose(1, 0, 2)
            .reshape(128, n_kt * n_out)
            .astype(np.float16)
        )

    common = {
        "basis_aug": basis_rep,
        "w0": pack_w(W0, KT1, HID),
        "w1": pack_w(W1, KT2, HID),
        "w2": pack_w(W2, KT2, HID),
        "w3": pack_w(W3, KT2, OUT),
        "b0t": np.asarray(b0, np.float32).reshape(MT_H, 128).T.copy(),
        "b1t": np.asarray(b1, np.float32).reshape(MT_H, 128).T.copy(),
        "b2t": np.asarray(b2, np.float32).reshape(MT_H, 128).T.copy(),
        "b3t": np.asarray(b3, np.float32).reshape(MT_O, 128).T.copy(),
    }
    in_maps = []
    for c in range(NCORES):
        m = dict(common)
        m["posT_aug"] = pos_aug_for_core(c)
        in_maps.append(m)
    return in_maps


def kernel(pos, basis, W0, b0, W1, b1, W2, b2, W3, b3, _trace=False):
    if "nc" not in _cache:
        _cache["nc"] = _build_program()
    nc = _cache["nc"]
    in_maps = _prep_inputs(pos, basis, W0, b0, W1, b1, W2, b2, W3, b3)
    res = run_bass_kernel_spmd(nc, in_maps, list(range(NCORES)), trace=_trace)
    _cache["last_result"] = res
    out = np.empty((B, OUT), np.float32)
    for c in range(NCORES):
        o = np.asarray(res.results[c]["outT"])  # [MT_O, 128, BPC]
        out[c * BPC : (c + 1) * BPC] = o.transpose(2, 0, 1).reshape(BPC, OUT)
    return out

